# revision 38
# baseline (speedup 1.0000x reference)
"""Trainium2 kernel for nn_CandidateFinder: LSH/Wu-Manber/Trie-masked top-64
candidate retrieval.

Math: for query (b,i) and key (b,j), the pair is a candidate iff
  sig-match:  binary sign-pattern of query_up[3,i] equals that of key_up[3,j]
  lsh-match:  lsh_hash(query_up[b,i]) == lsh_hash(key_up[b,j])
  inserted:   prefix-6 sign patterns of query_up[0,j] and key_up[0,j] agree
ranked by sims descending.  The sig-match condition is an exact 64-bit
pattern equality and is independent of the batch index, so the candidate set
of the whole [B,S,S] problem is empty unless some pair (i,j) of the single
[S,S] batch-3 sign-pattern problem matches exactly.

The device kernel decides that predicate exactly: with u = (x>0) - 0.5 in
{-0.5,+0.5} (bf16-exact, and exact reference semantics for x==0), the PE
computes z_ij = sum_d u_q[d,i] * u_k[d,j] over the 64 dims.  z is a
half-integer in [-16,16] accumulated exactly in fp32 PSUM, and z == 16 iff
the binary patterns agree on all 64 dims; any non-match gives z <= 15.5.
Each [128,1024] PSUM block is scanned by either the Activation engine
(Relu(z-15.625) with accum_out, sum > 0 iff suspicious) or the Vector engine
(reduce_max, >= 15.75 iff suspicious).  The 4096x4096 pair problem is
sharded 512 queries/core across 8 cores.  Queries and keys arrive
host-pre-transposed as one [64, 512+4096] array so no on-device transposes
are needed; staging DMA is split in three pieces so binarize/matmul/scan
pipeline behind it, and dummy PE/Act warm-up ops hide the PE p-state ramp
and the activation-table load.

The host reads back the 8x[128,16] accumulators: if nothing is suspicious,
no trie match exists anywhere, so the combined masks are all-false and the
reference output is exactly all -1.  Otherwise (needs an exact 64-bit
sign-pattern collision; probability ~0 for continuous inputs, and absent in
practice) the host recomputes the full exact answer in numpy.
"""

import os
import sys

for _p in ("/opt/trn_rl_repo", os.path.expanduser("~/.axon_site/_ro/trn_rl_repo")):
    if os.path.isdir(_p) and _p not in sys.path:
        sys.path.insert(0, _p)

import numpy as np

B, S, D, H = 4, 4096, 64, 16
K_MAX = 64
PREFIX_LEN = 6
LSH_BUCKETS = 64
LSH_BANDWIDTH = 4.0
NEG = np.float32(-1e30)

N_CORES = 8
QN = S // N_CORES        # 512 batch-3 query rows per core
KN = S                   # 4096 batch-3 key rows (replicated)
W_TOT = QN + KN          # merged [64, 4608] staged input

UNITS = 16               # scan units of [128, 1024] PSUM
# z = 16 iff exact 64-bit pattern match; non-match <= 15.5 (half-int grid)
THRESH = 15.75
RELU_BIAS = -15.625

_CACHE = {}


def _build_nc():
    import concourse.bacc as bacc
    import concourse.mybir as mybir
    from concourse.tile import TileContext

    dt = mybir.dt
    AF = mybir.ActivationFunctionType
    OP = mybir.AluOpType
    AX = mybir.AxisListType

    nc = bacc.Bacc("TRN2", target_bir_lowering=False, debug=False,
                   num_devices=N_CORES)

    qkt = nc.dram_tensor("qkt", [D, W_TOT], dt.bfloat16, kind="ExternalInput")
    accs = nc.dram_tensor("accs", [128, UNITS], dt.float32,
                          kind="ExternalOutput")

    QT = QN // 128           # 4 query tiles
    KC = KN // 1024          # 4 key chunks of 1024

    with TileContext(nc) as tc:
        with (
            tc.tile_pool(name="feat", bufs=1) as feat,
            tc.tile_pool(name="psz", bufs=4, space="PSUM") as psz,
        ):
            qkst = feat.tile([D, W_TOT], dt.bfloat16)
            fqk = feat.tile([D, W_TOT], dt.bfloat16)
            acc = feat.tile([128, UNITS], dt.float32)
            bias_t = feat.tile([128, 1], dt.float32)
            dummy_o = feat.tile([128, 1], dt.float32)
            warm_sb = feat.tile([D, 128], dt.bfloat16)

            # t~0: constants, act-table preload, PE p-state warm-up.  The
            # dummy activation forces the (Sign/Relu) table load before any
            # data arrives; the warm-up matmuls keep the PE ramping so the
            # real matmuls below run at full p-state.
            nc.gpsimd.memset(bias_t[:], RELU_BIAS)
            nc.gpsimd.memset(warm_sb[:], 0.0)
            nc.scalar.activation(dummy_o[:], bias_t[:], AF.Relu, bias=0.0)
            warm_pz = psz.tile([128, 1024], dt.float32, tag="pz")
            for _ in range(32):
                nc.tensor.matmul(warm_pz[:, 0:128], warm_sb[:], warm_sb[:],
                                 start=True, stop=True)

            # staging pieces: [fq | fk chunk0], [fk chunk1], [fk chunks 2-3]
            pieces = [(0, QN + 1024), (QN + 1024, 1024), (QN + 2048, 2048)]
            for off, w in pieces:
                nc.sync.dma_start(qkst[:, off:off + w], qkt[:, off:off + w])
            # binarize: piece 0 (fq + fk chunk0, gates all matmuls) on DVE
            # (4x bf16 mode), split so the first matmul's operands (fq +
            # keys 0..511) are ready one op earlier; later fk pieces on Pool
            nc.vector.tensor_scalar(fqk[:, 0:QN + 512], qkst[:, 0:QN + 512],
                                    0.0, 0.5, OP.is_gt, OP.subtract)
            nc.gpsimd.tensor_scalar(fqk[:, QN + 512:QN + 1024],
                                    qkst[:, QN + 512:QN + 1024],
                                    0.0, 0.5, OP.is_gt, OP.subtract)
            nc.gpsimd.tensor_scalar(fqk[:, QN + 1024:QN + 2048],
                                    qkst[:, QN + 1024:QN + 2048], 0.0, 0.5,
                                    OP.is_gt, OP.subtract)

            nc.gpsimd.tensor_scalar(fqk[:, QN + 2048:W_TOT],
                                    qkst[:, QN + 2048:W_TOT], 0.0, 0.5,
                                    OP.is_gt, OP.subtract)

            # main loop: kc-major so units follow staging availability.
            # Units alternate between the two PSUM-capable scanners, DVE
            # first (its queue is free a beat earlier than Act's).
            for kc in range(KC):
                for t in range(QT):
                    u = kc * QT + t
                    pz = psz.tile([128, 1024], dt.float32, tag="pz")
                    for n in range(2):
                        c0 = QN + kc * 1024 + n * 512
                        nc.tensor.matmul(pz[:, n * 512:(n + 1) * 512],
                                         fqk[:, t * 128:(t + 1) * 128],
                                         fqk[:, c0:c0 + 512],
                                         start=True, stop=True)
                    if u % 2 == 0:
                        nc.vector.reduce_max(acc[:, u:u + 1], pz[:], AX.X)
                    else:
                        nc.scalar.activation(pz[:], pz[:], AF.Relu,
                                             bias=bias_t[:],
                                             accum_out=acc[:, u:u + 1])

            nc.sync.dma_start(accs[:], acc[:])

    nc.compile()
    return nc


def _get_nc():
    if "nc" not in _CACHE:
        _CACHE["nc"] = _build_nc()
    return _CACHE["nc"]


def _reference_numpy(query_up, key_up, lsh_W):
    """Exact-semantics host fallback (needs a 64-bit sign collision; ~never)."""
    q = np.asarray(query_up, np.float32)
    k = np.asarray(key_up, np.float32)
    W = np.asarray(lsh_W, np.float32)
    qbin = (q > 0)
    kbin = (k > 0)

    def lsh_hash(x):
        proj = x.reshape(-1, D) @ W
        codes = np.floor(proj / LSH_BANDWIDTH).astype(np.int64)
        return (codes.sum(-1) % LSH_BUCKETS).reshape(B, S)

    qh = lsh_hash(q)
    kh = lsh_hash(k)
    inserted = np.all(qbin[0, :, :PREFIX_LEN] == kbin[0, :, :PREFIX_LEN], axis=-1)
    sig_match = np.all(qbin[-1][:, None, :] == kbin[-1][None, :, :], axis=-1)
    trie = sig_match & inserted[None, :]
    out = np.full((B, S, K_MAX), -1, np.int32)
    for b in range(B):
        lsh_m = qh[b][:, None] == kh[b][None, :]
        combined = lsh_m & trie
        sims = q[b] @ k[b].T
        masked = np.where(combined, sims, NEG)
        order = np.argsort(-masked, axis=-1, kind="stable")[:, :K_MAX]
        vals = np.take_along_axis(masked, order, axis=-1)
        out[b] = np.where(vals > NEG, order, -1).astype(np.int32)
    return out


def kernel(query_up, key_up, lsh_W, head_idx=0, **_):
    from concourse.bass_utils import run_bass_kernel_spmd

    q = np.asarray(query_up, np.float32)
    k = np.asarray(key_up, np.float32)
    W = np.asarray(lsh_W, np.float32)

    # the device receives bf16 inputs; sign((x>0)) survives the conversion
    # for every normal float, so only guard the tiny-denormal band.
    if np.any(np.abs(q[B - 1]) < 1e-38) or np.any(np.abs(k[B - 1]) < 1e-38):
        return _reference_numpy(q, k, W)

    import ml_dtypes
    qT = q[B - 1].T.astype(ml_dtypes.bfloat16)   # [64, 4096]
    kT = k[B - 1].T.astype(ml_dtypes.bfloat16)   # [64, 4096]

    in_maps = []
    for c in range(N_CORES):
        qk = np.empty((D, W_TOT), ml_dtypes.bfloat16)
        qk[:, :QN] = qT[:, c * QN:(c + 1) * QN]
        qk[:, QN:] = kT
        in_maps.append({"qkt": qk})

    nc = _get_nc()
    res = run_bass_kernel_spmd(nc, in_maps, list(range(N_CORES))).results

    # even slots hold per-unit max z (suspicious >= 15.75); odd slots hold
    # per-unit sum relu(z-15.625) (suspicious > 0).
    suspicious = False
    for c in range(N_CORES):
        a = res[c]["accs"]
        if float(a[:, 0::2].max()) >= THRESH or \
           float(a[:, 1::2].max()) > 0.05:
            suspicious = True
    if suspicious:
        return _reference_numpy(q, k, W)
    return np.full((B, S, K_MAX), -1, np.int32)


# revision 39
# speedup vs baseline: 1.0057x; 1.0057x over previous
"""Trainium2 kernel for nn_CandidateFinder: LSH/Wu-Manber/Trie-masked top-64
candidate retrieval.

Math: for query (b,i) and key (b,j), the pair is a candidate iff
  sig-match:  binary sign-pattern of query_up[3,i] equals that of key_up[3,j]
  lsh-match:  lsh_hash(query_up[b,i]) == lsh_hash(key_up[b,j])
  inserted:   prefix-6 sign patterns of query_up[0,j] and key_up[0,j] agree
ranked by sims descending.  The sig-match condition is an exact 64-bit
pattern equality and is independent of the batch index, so the candidate set
of the whole [B,S,S] problem is empty unless some pair (i,j) of the single
[S,S] batch-3 sign-pattern problem matches exactly.

The device kernel decides that predicate exactly: with u = (x>0) - 0.5 in
{-0.5,+0.5} (bf16-exact, and exact reference semantics for x==0), the PE
computes z_ij = sum_d u_q[d,i] * u_k[d,j] over the 64 dims.  z is a
half-integer in [-16,16] accumulated exactly in fp32 PSUM, and z == 16 iff
the binary patterns agree on all 64 dims; any non-match gives z <= 15.5.
Each [128,1024] PSUM block is scanned by either the Activation engine
(Relu(z-15.625) with accum_out, sum > 0 iff suspicious) or the Vector engine
(reduce_max, >= 15.75 iff suspicious).  The 4096x4096 pair problem is
sharded 512 queries/core across 8 cores.  Queries and keys arrive
host-pre-transposed as one [64, 512+4096] array so no on-device transposes
are needed; staging DMA is split in three pieces so binarize/matmul/scan
pipeline behind it, and dummy PE/Act warm-up ops hide the PE p-state ramp
and the activation-table load.

The host reads back the 8x[128,16] accumulators: if nothing is suspicious,
no trie match exists anywhere, so the combined masks are all-false and the
reference output is exactly all -1.  Otherwise (needs an exact 64-bit
sign-pattern collision; probability ~0 for continuous inputs, and absent in
practice) the host recomputes the full exact answer in numpy.
"""

import os
import sys

for _p in ("/opt/trn_rl_repo", os.path.expanduser("~/.axon_site/_ro/trn_rl_repo")):
    if os.path.isdir(_p) and _p not in sys.path:
        sys.path.insert(0, _p)

import numpy as np

B, S, D, H = 4, 4096, 64, 16
K_MAX = 64
PREFIX_LEN = 6
LSH_BUCKETS = 64
LSH_BANDWIDTH = 4.0
NEG = np.float32(-1e30)

N_CORES = 8
QN = S // N_CORES        # 512 batch-3 query rows per core
KN = S                   # 4096 batch-3 key rows (replicated)
W_TOT = QN + KN          # merged [64, 4608] staged input

UNITS = 16               # scan units of [128, 1024] PSUM
# z = 16 iff exact 64-bit pattern match; non-match <= 15.5 (half-int grid)
THRESH = 15.75
RELU_BIAS = -15.625

_CACHE = {}


def _build_nc():
    import concourse.bacc as bacc
    import concourse.mybir as mybir
    from concourse.tile import TileContext

    dt = mybir.dt
    AF = mybir.ActivationFunctionType
    OP = mybir.AluOpType
    AX = mybir.AxisListType

    nc = bacc.Bacc("TRN2", target_bir_lowering=False, debug=False,
                   num_devices=N_CORES)

    qkt = nc.dram_tensor("qkt", [D, W_TOT], dt.bfloat16, kind="ExternalInput")
    accs = nc.dram_tensor("accs", [128, UNITS], dt.float32,
                          kind="ExternalOutput")

    QT = QN // 128           # 4 query tiles
    KC = KN // 1024          # 4 key chunks of 1024

    with TileContext(nc) as tc:
        with (
            tc.tile_pool(name="feat", bufs=1) as feat,
            tc.tile_pool(name="psz", bufs=4, space="PSUM") as psz,
        ):
            qkst = feat.tile([D, W_TOT], dt.bfloat16)
            fqk = feat.tile([D, W_TOT], dt.bfloat16)
            acc = feat.tile([128, UNITS], dt.float32)
            bias_t = feat.tile([128, 1], dt.float32)
            dummy_o = feat.tile([128, 1], dt.float32)
            warm_sb = feat.tile([D, 128], dt.bfloat16)

            # t~0: constants, act-table preload, PE p-state warm-up.  The
            # dummy activation forces the (Sign/Relu) table load before any
            # data arrives; the warm-up matmuls keep the PE ramping so the
            # real matmuls below run at full p-state.
            nc.gpsimd.memset(bias_t[:], RELU_BIAS)
            nc.vector.memset(warm_sb[:], 0.0)
            nc.scalar.activation(dummy_o[:], bias_t[:], AF.Relu, bias=0.0)
            warm_pz = psz.tile([128, 1024], dt.float32, tag="pz")
            for _ in range(31):
                nc.tensor.matmul(warm_pz[:, 0:128], warm_sb[:], warm_sb[:],
                                 start=True, stop=True)

            # staging pieces: [fq | fk chunk0], [fk chunk1], [fk chunks 2-3]
            pieces = [(0, QN + 1024), (QN + 1024, 1024), (QN + 2048, 2048)]
            for off, w in pieces:
                nc.sync.dma_start(qkst[:, off:off + w], qkt[:, off:off + w])
            # binarize: piece 0 (fq + fk chunk0, gates all matmuls) on DVE
            # (4x bf16 mode), split so the first matmul's operands (fq +
            # keys 0..511) are ready one op earlier; later fk pieces on Pool
            nc.vector.tensor_scalar(fqk[:, 0:QN + 512], qkst[:, 0:QN + 512],
                                    0.0, 0.5, OP.is_gt, OP.subtract)
            nc.gpsimd.tensor_scalar(fqk[:, QN + 512:QN + 1024],
                                    qkst[:, QN + 512:QN + 1024],
                                    0.0, 0.5, OP.is_gt, OP.subtract)
            nc.gpsimd.tensor_scalar(fqk[:, QN + 1024:QN + 2048],
                                    qkst[:, QN + 1024:QN + 2048], 0.0, 0.5,
                                    OP.is_gt, OP.subtract)

            nc.gpsimd.tensor_scalar(fqk[:, QN + 2048:W_TOT],
                                    qkst[:, QN + 2048:W_TOT], 0.0, 0.5,
                                    OP.is_gt, OP.subtract)

            # main loop: kc-major so units follow staging availability.
            # Units alternate between the two PSUM-capable scanners, DVE
            # first (its queue is free a beat earlier than Act's).
            for kc in range(KC):
                for t in range(QT):
                    u = kc * QT + t
                    pz = psz.tile([128, 1024], dt.float32, tag="pz")
                    for n in range(2):
                        c0 = QN + kc * 1024 + n * 512
                        nc.tensor.matmul(pz[:, n * 512:(n + 1) * 512],
                                         fqk[:, t * 128:(t + 1) * 128],
                                         fqk[:, c0:c0 + 512],
                                         start=True, stop=True)
                    if u % 2 == 0:
                        nc.vector.reduce_max(acc[:, u:u + 1], pz[:], AX.X)
                    else:
                        nc.scalar.activation(pz[:], pz[:], AF.Relu,
                                             bias=bias_t[:],
                                             accum_out=acc[:, u:u + 1])

            nc.sync.dma_start(accs[:], acc[:])

    nc.compile()
    return nc


def _get_nc():
    if "nc" not in _CACHE:
        _CACHE["nc"] = _build_nc()
    return _CACHE["nc"]


def _reference_numpy(query_up, key_up, lsh_W):
    """Exact-semantics host fallback (needs a 64-bit sign collision; ~never)."""
    q = np.asarray(query_up, np.float32)
    k = np.asarray(key_up, np.float32)
    W = np.asarray(lsh_W, np.float32)
    qbin = (q > 0)
    kbin = (k > 0)

    def lsh_hash(x):
        proj = x.reshape(-1, D) @ W
        codes = np.floor(proj / LSH_BANDWIDTH).astype(np.int64)
        return (codes.sum(-1) % LSH_BUCKETS).reshape(B, S)

    qh = lsh_hash(q)
    kh = lsh_hash(k)
    inserted = np.all(qbin[0, :, :PREFIX_LEN] == kbin[0, :, :PREFIX_LEN], axis=-1)
    sig_match = np.all(qbin[-1][:, None, :] == kbin[-1][None, :, :], axis=-1)
    trie = sig_match & inserted[None, :]
    out = np.full((B, S, K_MAX), -1, np.int32)
    for b in range(B):
        lsh_m = qh[b][:, None] == kh[b][None, :]
        combined = lsh_m & trie
        sims = q[b] @ k[b].T
        masked = np.where(combined, sims, NEG)
        order = np.argsort(-masked, axis=-1, kind="stable")[:, :K_MAX]
        vals = np.take_along_axis(masked, order, axis=-1)
        out[b] = np.where(vals > NEG, order, -1).astype(np.int32)
    return out


def kernel(query_up, key_up, lsh_W, head_idx=0, **_):
    from concourse.bass_utils import run_bass_kernel_spmd

    q = np.asarray(query_up, np.float32)
    k = np.asarray(key_up, np.float32)
    W = np.asarray(lsh_W, np.float32)

    # the device receives bf16 inputs; sign((x>0)) survives the conversion
    # for every normal float, so only guard the tiny-denormal band.
    if np.any(np.abs(q[B - 1]) < 1e-38) or np.any(np.abs(k[B - 1]) < 1e-38):
        return _reference_numpy(q, k, W)

    import ml_dtypes
    qT = q[B - 1].T.astype(ml_dtypes.bfloat16)   # [64, 4096]
    kT = k[B - 1].T.astype(ml_dtypes.bfloat16)   # [64, 4096]

    in_maps = []
    for c in range(N_CORES):
        qk = np.empty((D, W_TOT), ml_dtypes.bfloat16)
        qk[:, :QN] = qT[:, c * QN:(c + 1) * QN]
        qk[:, QN:] = kT
        in_maps.append({"qkt": qk})

    nc = _get_nc()
    res = run_bass_kernel_spmd(nc, in_maps, list(range(N_CORES))).results

    # even slots hold per-unit max z (suspicious >= 15.75); odd slots hold
    # per-unit sum relu(z-15.625) (suspicious > 0).
    suspicious = False
    for c in range(N_CORES):
        a = res[c]["accs"]
        if float(a[:, 0::2].max()) >= THRESH or \
           float(a[:, 1::2].max()) > 0.05:
            suspicious = True
    if suspicious:
        return _reference_numpy(q, k, W)
    return np.full((B, S, K_MAX), -1, np.int32)
